# revision 2
# baseline (speedup 1.0000x reference)
"""DensityExtractor (NeRF volume-rendering weights) Bass kernel for 8 TRN2 cores.

reference:
  dists[s] = d[s+1]-d[s] (last 1e10), scaled by |ray_dir|
  alpha = 1 - exp(-relu(rf[...,3]) * dists)
  weights = alpha * cumprod_exclusive(1 - alpha + 1e-10)

Strategy: fully data-parallel over rays (65536 -> 8 x 8192). Everything stays
in layout A ([ray partition, sample free]); the per-ray cumprod uses the DVE
tensor_tensor_scan instruction (prefix scan along the free dim), so there are
no transposes, no PSUM, no TensorE use at all.

Per block of 512 rays (4 groups of 128, packed [128, 512] along free):
  dd   = shifted diff of packed depth; group-boundary cols overwritten w/ 1e10
  m    = (sigma max 0) * dd               -- one fused scalar_tensor_tensor,
                                             sigma read strided (ch 3 of rf)
  e    = exp(-dn * m)                     -- 4 ACT ops, per-partition scale
                                             folds in dir_norm (cumsum(dn*m)
                                             == dn*cumsum(m) trick not needed:
                                             exp runs pre-scan elementwise)
  T    = scan: state = e*state + 1e-10    -- inclusive cumprod, 4 scans
         (ref recurrence is (e+1e-10)*state; diff <= 1e-10*(1-state) < 2e-8)
  w    = (T[s-1] + 1e-10) - T[s]          -- == alpha * T_exclusive exactly
         boundary cols s=0: w = (1+1e-10) - T[s=0]
"""

import sys

for _p in ("/opt/trn_rl_repo", "/root/.axon_site/_ro/trn_rl_repo"):
    if _p not in sys.path:
        sys.path.append(_p)

from contextlib import ExitStack

import numpy as np

import concourse.bass as bass
import concourse.tile as tile
from concourse import bacc, mybir
from concourse.bass_utils import run_bass_kernel_spmd

FP = mybir.dt.float32
OP = mybir.AluOpType
AF = mybir.ActivationFunctionType
N_CORES = 8
N_RAYS = 65536
S = 128
BLK = 512  # rays per block
ONE_E_10 = 1.0e10
EPS = 1.0e-10


def build_module(n_rays=N_RAYS // N_CORES, scan_split=4, bufs=3):
    """scan_split: how many of the 4 per-block scans run on DVE (rest POOL)."""
    nblk = n_rays // BLK
    assert n_rays % BLK == 0
    ntile = n_rays // 128

    nc = bacc.Bacc("TRN2", target_bir_lowering=False, debug=False)
    rf = nc.dram_tensor("radiance_field", [n_rays, S, 4], FP, kind="ExternalInput").ap()
    dv = nc.dram_tensor("depth_values", [n_rays, S], FP, kind="ExternalInput").ap()
    rd = nc.dram_tensor("ray_directions", [n_rays, 3], FP, kind="ExternalInput").ap()
    out = nc.dram_tensor("weights", [n_rays, S], FP, kind="ExternalOutput").ap()

    with tile.TileContext(nc) as tc, ExitStack() as ctx:
        consts = ctx.enter_context(tc.tile_pool(name="consts", bufs=1))
        rfp = ctx.enter_context(tc.tile_pool(name="rf", bufs=bufs))
        dvp = ctx.enter_context(tc.tile_pool(name="dv", bufs=bufs))
        ddp = ctx.enter_context(tc.tile_pool(name="dd", bufs=bufs))
        mpp = ctx.enter_context(tc.tile_pool(name="m", bufs=bufs))
        epp = ctx.enter_context(tc.tile_pool(name="e", bufs=bufs))
        tpp = ctx.enter_context(tc.tile_pool(name="T", bufs=bufs))
        wpp = ctx.enter_context(tc.tile_pool(name="w", bufs=bufs))

        # epsilon tile for the scan's add operand
        eps_t = consts.tile([128, S], FP, tag="eps")
        nc.gpsimd.memset(eps_t[:], EPS)

        # --- dir_norm prologue: dnneg[j, t] = -|ray_dir| of ray 128*t + j ---
        rdt = consts.tile([128, ntile * 3], FP, tag="rdt")
        nc.sync.dma_start(
            rdt[:].rearrange("j (t c) -> j t c", c=3),
            rd.rearrange("(t j) c -> j t c", j=128),
        )
        sq = consts.tile([128, ntile * 3], FP, tag="sq")
        nc.vector.tensor_mul(sq[:], rdt[:], rdt[:])
        sq3 = sq[:].rearrange("j (t c) -> j t c", c=3)
        dn2 = consts.tile([128, ntile], FP, tag="dn2")
        nc.vector.tensor_add(dn2[:], sq3[:, :, 0], sq3[:, :, 1])
        nc.vector.tensor_add(dn2[:], dn2[:], sq3[:, :, 2])
        dnneg = consts.tile([128, ntile], FP, tag="dnneg")
        nc.scalar.activation(dnneg[:], dn2[:], AF.Sqrt)
        nc.vector.tensor_scalar_mul(dnneg[:], dnneg[:], -1.0)

        for b in range(nblk):
            r0 = b * BLK
            # one contiguous DMA each; packed [128 rays, 4 groups x free]
            rf_p = rfp.tile([128, 4 * S * 4], FP, tag="rf")
            nc.sync.dma_start(
                rf_p[:].rearrange("r (q f) -> r q f", q=4),
                rf[r0 : r0 + BLK].rearrange("(q r) s c -> r q (s c)", q=4),
            )
            dv_p = dvp.tile([128, 4 * S], FP, tag="dv")
            nc.sync.dma_start(
                dv_p[:].rearrange("r (q s) -> r q s", q=4),
                dv[r0 : r0 + BLK, :].rearrange("(q r) s -> r q s", q=4),
            )

            # dd: shifted diff; cross-group garbage cols are exactly the ones
            # that must become 1e10
            dd = ddp.tile([128, 4 * S], FP, tag="dd")
            nc.gpsimd.tensor_sub(dd[:, 0 : 4 * S - 1], dv_p[:, 1:], dv_p[:, : 4 * S - 1])
            nc.gpsimd.memset(
                dd[:].rearrange("r (q s) -> r q s", q=4)[:, :, S - 1], ONE_E_10
            )

            # m = relu(sigma) * dd, sigma strided from packed rf
            sig = rf_p[:].rearrange("r (x c) -> r x c", c=4)[:, :, 3]
            m_ = mpp.tile([128, 4 * S], FP, tag="m")
            nc.vector.scalar_tensor_tensor(m_[:], sig, 0.0, dd[:], OP.max, OP.mult)

            # e = exp(-dn * m), per ray-tile (per-partition scale)
            e_ = epp.tile([128, 4 * S], FP, tag="e")
            for q in range(4):
                sl = slice(S * q, S * (q + 1))
                t_idx = 4 * b + q
                nc.scalar.activation(
                    e_[:, sl], m_[:, sl], AF.Exp, scale=dnneg[:, t_idx : t_idx + 1]
                )

            # inclusive cumprod via scan: state = e*state + 1e-10
            T_ = tpp.tile([128, 4 * S], FP, tag="T")
            for q in range(4):
                sl = slice(S * q, S * (q + 1))
                eng = nc.vector if q < scan_split else nc.gpsimd
                eng.tensor_tensor_scan(
                    T_[:, sl], e_[:, sl], eps_t[:], 1.0, OP.mult, OP.add
                )

            # w[s] = (T[s-1] + 1e-10) - T[s]; boundary cols w = (1+1e-10) - T
            w_ = wpp.tile([128, 4 * S], FP, tag="w")
            nc.vector.scalar_tensor_tensor(
                w_[:, 1:], T_[:, : 4 * S - 1], EPS, T_[:, 1:], OP.add, OP.subtract
            )
            w3 = w_[:].rearrange("r (q s) -> r q s", q=4)[:, :, 0]
            T3 = T_[:].rearrange("r (q s) -> r q s", q=4)[:, :, 0]
            nc.vector.tensor_scalar(w3, T3, -1.0, 1.0 + EPS, OP.mult, OP.add)

            nc.sync.dma_start(
                out[r0 : r0 + BLK, :].rearrange("(q r) s -> r q s", q=4),
                w_[:].rearrange("r (q s) -> r q s", q=4),
            )

    nc.compile()
    return nc


_NC_CACHE = {}


def get_module(n_rays=N_RAYS // N_CORES):
    if n_rays not in _NC_CACHE:
        _NC_CACHE[n_rays] = build_module(n_rays)
    return _NC_CACHE[n_rays]


def run_spmd(radiance_field, depth_values, ray_directions, trace=False):
    nc = get_module()
    per = radiance_field.shape[0] // N_CORES
    in_maps = []
    for i in range(N_CORES):
        s = slice(i * per, (i + 1) * per)
        in_maps.append(
            {
                "radiance_field": np.ascontiguousarray(radiance_field[s]),
                "depth_values": np.ascontiguousarray(depth_values[s]),
                "ray_directions": np.ascontiguousarray(ray_directions[s]),
            }
        )
    res = run_bass_kernel_spmd(nc, in_maps, list(range(N_CORES)), trace=trace)
    out = np.concatenate([r["weights"] for r in res.results], axis=0)
    return out, res


def kernel(radiance_field, depth_values, ray_directions):
    out, _ = run_spmd(
        np.asarray(radiance_field, dtype=np.float32),
        np.asarray(depth_values, dtype=np.float32),
        np.asarray(ray_directions, dtype=np.float32),
    )
    return out


# revision 4
# speedup vs baseline: 1.0356x; 1.0356x over previous
"""DensityExtractor (NeRF volume-rendering weights) Bass kernel for 8 TRN2 cores.

reference:
  dists[s] = d[s+1]-d[s] (last 1e10), scaled by |ray_dir|
  alpha = 1 - exp(-relu(rf[...,3]) * dists)
  weights = alpha * cumprod_exclusive(1 - alpha + 1e-10)

Strategy: fully data-parallel over rays (65536 -> 8 x 8192). Everything stays
in layout [ray partition, sample free]; the per-ray cumprod uses the
tensor_tensor_scan instruction (prefix scan along the free dim), so there are
no transposes, no PSUM, no TensorE use at all.

Blocks of 512 rays packed [128, 512] with ray = 512*b + 4*r + k (partition r,
group k): each partition's DMA run is 4 consecutive rays -> 8KB contiguous for
rf, 2KB for depth/weights.

  dd   = shifted diff of packed depth; group-boundary cols overwritten w/ 1e10
  m    = (sigma max 0) * dd               -- fused scalar_tensor_tensor,
                                             sigma read strided (ch 3 of rf)
  e    = exp(-dn * m)                     -- 4 ACT ops, per-partition scale
                                             folds in dir_norm
  T    = scan: state = e*state + 1e-10    -- inclusive cumprod per group
         (ref recurrence is (e+1e-10)*state; diff <= 1e-10*(1-state) < 2e-8)
  w    = (T[s-1] + 1e-10) - T[s]          -- == alpha * T_exclusive exactly
         group-boundary cols s=0: w = (1+1e-10) - T[s=0]
"""

import sys

for _p in ("/opt/trn_rl_repo", "/root/.axon_site/_ro/trn_rl_repo"):
    if _p not in sys.path:
        sys.path.append(_p)

from contextlib import ExitStack

import numpy as np

import concourse.bass as bass
import concourse.tile as tile
from concourse import bacc, mybir
from concourse.bass_utils import run_bass_kernel_spmd

FP = mybir.dt.float32
OP = mybir.AluOpType
AF = mybir.ActivationFunctionType
N_CORES = 8
N_RAYS = 65536
S = 128
BLK = 512  # rays per block
ONE_E_10 = 1.0e10
EPS = 1.0e-10


def build_module(n_rays=N_RAYS // N_CORES, dve_scans=2, bufs=3):
    """dve_scans: how many of the 4 per-block scans run on DVE (rest POOL)."""
    nblk = n_rays // BLK
    assert n_rays % BLK == 0

    nc = bacc.Bacc("TRN2", target_bir_lowering=False, debug=False)
    rf = nc.dram_tensor("radiance_field", [n_rays, S, 4], FP, kind="ExternalInput").ap()
    dv = nc.dram_tensor("depth_values", [n_rays, S], FP, kind="ExternalInput").ap()
    rd = nc.dram_tensor("ray_directions", [n_rays, 3], FP, kind="ExternalInput").ap()
    out = nc.dram_tensor("weights", [n_rays, S], FP, kind="ExternalOutput").ap()

    with tile.TileContext(nc) as tc, ExitStack() as ctx:
        consts = ctx.enter_context(tc.tile_pool(name="consts", bufs=1))
        rfp = ctx.enter_context(tc.tile_pool(name="rf", bufs=bufs))
        dvp = ctx.enter_context(tc.tile_pool(name="dv", bufs=bufs))
        ddp = ctx.enter_context(tc.tile_pool(name="dd", bufs=bufs))
        mpp = ctx.enter_context(tc.tile_pool(name="m", bufs=bufs))
        epp = ctx.enter_context(tc.tile_pool(name="e", bufs=bufs))
        tpp = ctx.enter_context(tc.tile_pool(name="T", bufs=bufs))
        wpp = ctx.enter_context(tc.tile_pool(name="w", bufs=bufs))

        # epsilon tile for the scan's add operand
        eps_t = consts.tile([128, S], FP, tag="eps")
        nc.gpsimd.memset(eps_t[:], EPS)

        # --- dir_norm prologue ---
        # dnneg[r, 4b+k] = -|ray_dir| of ray 512b + 4r + k (matches block
        # packing below). Per-partition DMA runs are 48B (4 rays' xyz).
        rdt = consts.tile([128, nblk * 12], FP, tag="rdt")
        nc.sync.dma_start(
            rdt[:].rearrange("r (b k c) -> r b k c", b=nblk, k=4),
            rd.rearrange("(b r k) c -> r b k c", r=128, k=4),
        )
        sq = consts.tile([128, nblk * 12], FP, tag="sq")
        nc.vector.tensor_mul(sq[:], rdt[:], rdt[:])
        sq3 = sq[:].rearrange("r (t c) -> r t c", c=3)
        dn2 = consts.tile([128, nblk * 4], FP, tag="dn2")
        nc.vector.tensor_add(dn2[:], sq3[:, :, 0], sq3[:, :, 1])
        nc.vector.tensor_add(dn2[:], dn2[:], sq3[:, :, 2])
        dnneg = consts.tile([128, nblk * 4], FP, tag="dnneg")
        nc.scalar.activation(dnneg[:], dn2[:], AF.Sqrt)
        nc.vector.tensor_scalar_mul(dnneg[:], dnneg[:], -1.0)

        for b in range(nblk):
            r0 = b * BLK
            # one contiguous DMA each; ray = r0 + 4*r + k
            rf_p = rfp.tile([128, 4 * S * 4], FP, tag="rf")
            nc.sync.dma_start(
                rf_p[:].rearrange("r (k f) -> r k f", k=4),
                rf[r0 : r0 + BLK].rearrange("(r k) s c -> r k (s c)", k=4),
            )
            dv_p = dvp.tile([128, 4 * S], FP, tag="dv")
            nc.sync.dma_start(
                dv_p[:].rearrange("r (k s) -> r k s", k=4),
                dv[r0 : r0 + BLK, :].rearrange("(r k) s -> r k s", k=4),
            )

            # dd: shifted diff; cross-group garbage cols are exactly the ones
            # that must become 1e10
            dd = ddp.tile([128, 4 * S], FP, tag="dd")
            nc.gpsimd.tensor_sub(dd[:, 0 : 4 * S - 1], dv_p[:, 1:], dv_p[:, : 4 * S - 1])
            nc.gpsimd.memset(
                dd[:].rearrange("r (k s) -> r k s", k=4)[:, :, S - 1], ONE_E_10
            )

            # m = relu(sigma) * dd, sigma strided from packed rf
            sig = rf_p[:].rearrange("r (x c) -> r x c", c=4)[:, :, 3]
            m_ = mpp.tile([128, 4 * S], FP, tag="m")
            nc.vector.scalar_tensor_tensor(m_[:], sig, 0.0, dd[:], OP.max, OP.mult)

            # e = exp(-dn * m), per group k (per-partition scale)
            e_ = epp.tile([128, 4 * S], FP, tag="e")
            for k in range(4):
                sl = slice(S * k, S * (k + 1))
                c = 4 * b + k
                nc.scalar.activation(
                    e_[:, sl], m_[:, sl], AF.Exp, scale=dnneg[:, c : c + 1]
                )

            # inclusive cumprod via scan: state = e*state + 1e-10 (DVE-only op)
            T_ = tpp.tile([128, 4 * S], FP, tag="T")
            for k in range(4):
                sl = slice(S * k, S * (k + 1))
                nc.vector.tensor_tensor_scan(
                    T_[:, sl], e_[:, sl], eps_t[:], 1.0, OP.mult, OP.add
                )

            # w[s] = T[s-1] - T[s]  (== alpha*T_exc - 1e-10; the 1e-10 offset
            # is far below tolerance); boundary cols w = (1+1e-10) - T[s=0]
            w_ = wpp.tile([128, 4 * S], FP, tag="w")
            nc.gpsimd.tensor_sub(w_[:, 1:], T_[:, : 4 * S - 1], T_[:, 1:])
            w3 = w_[:].rearrange("r (k s) -> r k s", k=4)[:, :, 0]
            T3 = T_[:].rearrange("r (k s) -> r k s", k=4)[:, :, 0]
            nc.vector.tensor_scalar(w3, T3, -1.0, 1.0 + EPS, OP.mult, OP.add)

            nc.sync.dma_start(
                out[r0 : r0 + BLK, :].rearrange("(r k) s -> r k s", k=4),
                w_[:].rearrange("r (k s) -> r k s", k=4),
            )

    nc.compile()
    return nc


_NC_CACHE = {}


def get_module(n_rays=N_RAYS // N_CORES, **kw):
    key = (n_rays, tuple(sorted(kw.items())))
    if key not in _NC_CACHE:
        _NC_CACHE[key] = build_module(n_rays, **kw)
    return _NC_CACHE[key]


def run_spmd(radiance_field, depth_values, ray_directions, trace=False, **kw):
    nc = get_module(**kw)
    per = radiance_field.shape[0] // N_CORES
    in_maps = []
    for i in range(N_CORES):
        s = slice(i * per, (i + 1) * per)
        in_maps.append(
            {
                "radiance_field": np.ascontiguousarray(radiance_field[s]),
                "depth_values": np.ascontiguousarray(depth_values[s]),
                "ray_directions": np.ascontiguousarray(ray_directions[s]),
            }
        )
    res = run_bass_kernel_spmd(nc, in_maps, list(range(N_CORES)), trace=trace)
    out = np.concatenate([r["weights"] for r in res.results], axis=0)
    return out, res


def kernel(radiance_field, depth_values, ray_directions):
    out, _ = run_spmd(
        np.asarray(radiance_field, dtype=np.float32),
        np.asarray(depth_values, dtype=np.float32),
        np.asarray(ray_directions, dtype=np.float32),
    )
    return out
